# revision 39
# baseline (speedup 1.0000x reference)
"""Trainium2 Bass kernel for nn_Kernel_503460026817608471_12541304504791.

Full-input contract: kernel(x=(64,512,512) f32, p1_w=(512,1) f32,
p7_w=(1,512,512) f32) -> (64,512,512) f32.

Strategy: pure data parallel over the batch dim (8 items per NeuronCore).
Per batch item n (C = H = 512, P = 128, 4 partition chunks per matrix):

  t1  = p1.x          (PE matvec, [1,512])
  t2  = roll(x,2,c)   (C-layout only: row-shifted DMA loads from HBM)
  t3  = t1 + t2       (H: per-partition bias add; C: ones-matmul broadcast)
  t4  = x*t3, t5 = relu(x), t7 = p7*t5, t11 = min(t4, x)
  9 512^3 matmuls. C-side level-1 matmuls (t8, t13T) take f32r operands;
  the H side and everything downstream is bf16.

v2 structural changes over the v1 baseline:
  * t10 and t14 are never materialized untransposed: MM3/MM7 emit the
    TRANSPOSED logits directly by swapping lhsT/rhs
      (t7 t3^T)^T = t3 t7^T   -> mm512(t3h, t7h) = E10T logits
      (t9^T t10^T)^T = t10 t9 -> mm512(E10T, t9) = E14T logits
    which deletes 32 PE transposes + 8 ACT evacuations per item.  The
    softmax row-normalizers (now along the PSUM partition axis) are
    recomputed cheaply:
      Z10 rides MM7 itself: e10T is MM7's STATIONARY operand, so one
      extra N=1 ones-rhs matmul per (m,k) accumulates Z10 directly in
      per-partition column form (no streaming, no transposes);
      rr10c = INV_SC*INV_SH / Z10 then rides MM7's exp-evac scale.
      Z14 via 4 ones-lhsT reduce matmuls -> [1,512] row -> recip ->
      4 tiny PE transposes a STAGE LATER (s3) so the in-order PE queue
      never waits; rr14c = INV_SH / Z14 rides MM9's evac scale.
  * MM7 runs as fp8e4 DoubleRow (half the streaming): e10T entries are
    ~1 +/- 0.1 and t14's softmax is nearly flat, so quantizing e10T and
    a separate fp8 shadow of t9 measures 5.5e-5 end-to-end error; HW
    A/B: 921us vs 943us (bf16) per 8-item batch.
  * H-side tensors (xh/t5h/t3h/t4h/t7h, p7h) are bf16: the f32r PE
    transpose of x evacuates straight to bf16; DVE elementwise gets
    2-4x mode; MM2/MM3 take bf16 operands.

HW notes (8 axon-tunneled cores, no NTFF profiling): CoreSim predicts
260us/core but HW measures ~921us per 8-item batch (structural ~3.5x
gap: dispatch/DMA/sem costs underestimated).  CoreSim-guided changes
MUST be A/B'd on HW via bench_hw.py's loop-delta method — e.g. ONE
gpsimd tensor_reduce per item measured +0.5ms/batch (sim: 0.85us) and
is disabled (USE_GPRED=0).  HW A/B results: v1 baseline 965us, this
version 921us/batch.

Softmaxes: inputs are provably in ~[-5,5], so exp runs without the
max-subtraction pass.  Remaining normalizer folds (unchanged from v1):
  t8 rows  -> scale on t13T's MM5 evacuation (same contraction index)
  t15 rows -> scale on t16's MM8 evacuation

Engine balance: every PSUM read goes through the ACT engine (DVE reads
from PSUM are far slower on HW than modeled).  GPSIMD (Pool) takes
SBUF-only relu/bias work off the DVE.  All scale factors ride
evacuation scales or softmax exp scales.

Software pipeline: each item splits into S1 (loads, elementwise,
MM1/MM2/MM3/MM5 + Z10 chain), S2 (MM4/MM7/MM6 + Z14 chain), S3
(MM8/MM9 + store), emitted stage-shifted [S1(k), S2(k-1), S3(k-2)].
"""

import math
import os
import sys

for _p in ("/opt/trn_rl_repo",):
    if _p not in sys.path and os.path.isdir(_p):
        sys.path.insert(0, _p)

import numpy as np

import concourse.bass as bass
import concourse.tile as tile
from concourse import bacc
from concourse import mybir
from concourse import bass_utils
from concourse.masks import make_identity

N, C, H = 64, 512, 512
SHIFT = 2
NCORES = 8
NB = N // NCORES          # batch items per core
P = 128
KC = C // P               # 4 chunks along c
KH = H // P               # 4 chunks along h

F32 = mybir.dt.float32
F32R = mybir.dt.float32r
BF16 = mybir.dt.bfloat16
F8 = mybir.dt.float8e4
AF = mybir.ActivationFunctionType
ALU = mybir.AluOpType
AX = mybir.AxisListType
PM = mybir.MatmulPerfMode

MDT = F32R                # C-side level-1 matmul operand dtype
BDT = BF16                # H-side + downstream operand dtype
USE_GP = os.environ.get("BASS_GP", "1") == "1"       # offload to GPSIMD
USE_XHB = os.environ.get("BASS_XHB", "0") == "1"     # xh via DMA bounce
# MM7 as fp8e4 DoubleRow: e10T entries are ~1+/-0.1 and t14's softmax is
# nearly flat, so quantizing e10T + a SEPARATE fp8 copy of t9 for this
# one matmul measures 5.5e-5 end-to-end error (numpy e4m3 experiment) -
# the double-softmax structure absorbs it.  Halves MM7's PE time.
USE_FP8 = os.environ.get("BASS_FP8", "1") == "1"
# gpsimd tensor_reduce measured ~+0.5ms/batch on HW (CoreSim models it
# ~0.85us); default to the PE ones-matmul reduce for Z14.
USE_GPRED = os.environ.get("BASS_GPRED", "0") == "1"

INV_SC = 1.0 / math.sqrt(C)
INV_SH = 1.0 / math.sqrt(H)


def _f(ap):
    """Read f32 view of an f32r tile for elementwise ops."""
    return ap.bitcast(F32) if ap.dtype == F32R else ap


def build_program(loop=1):
    """loop>1 re-emits the item pipeline `loop` times (same data) inside
    one NEFF - used only by bench_hw.py to measure steady-state HW time
    as (wall(K) - wall(1)) / (K - 1), cancelling dispatch overhead."""
    nc = bacc.Bacc("TRN2", target_bir_lowering=False, debug=False,
                   num_devices=NCORES)
    x = nc.dram_tensor("x", [NB, C, H], F32, kind="ExternalInput").ap()
    p1 = nc.dram_tensor("p1", [C, 1], F32, kind="ExternalInput").ap()
    p7 = nc.dram_tensor("p7", [C, H], F32, kind="ExternalInput").ap()
    out = nc.dram_tensor("out", [NB, C, H], F32, kind="ExternalOutput").ap()

    with tile.TileContext(nc) as tc:
        _build_body(nc, tc, x, p1, p7, out, loop=loop)
    nc.compile()
    return nc


def _build_body(nc, tc, x, p1, p7, out, loop=1):
    from contextlib import ExitStack
    ctx = ExitStack()
    constp = ctx.enter_context(tc.tile_pool(name="const", bufs=1))
    big = ctx.enter_context(tc.tile_pool(name="big", bufs=1))
    # double-buffered: tensors read late in item n whose rewrite would
    # otherwise stall item n+1's level-1 matmuls
    big2 = ctx.enter_context(tc.tile_pool(name="big2", bufs=2))
    xpool = ctx.enter_context(tc.tile_pool(name="xc", bufs=2))
    small = ctx.enter_context(tc.tile_pool(name="small", bufs=2))
    # stage-1-internal scalars: no cross-stage lifetime, single buffer
    small1 = ctx.enter_context(tc.tile_pool(name="small1", bufs=1))
    # PSUM budget (8 banks): g1 [P,512]x6 = 6, psmisc = 1, psbc = 1.
    # A deep g1 rotation lets fast small PE ops (xh transposes) run well
    # ahead of their trailing ACT evacuations at item boundaries; merged
    # [P,1024] evacs were tried instead and lost more to slot stalls than
    # they saved on ACT per-op overhead.
    psmm = ctx.enter_context(
        tc.tile_pool(name="psmm", bufs=6, space="PSUM"))
    psmisc = ctx.enter_context(
        tc.tile_pool(name="psmisc", bufs=1, space="PSUM"))

    gp = nc.gpsimd if USE_GP else nc.vector

    def relu(dst, src):
        """t5 = relu(x) via tensor_scalar max-with-0 (no ACT tables)."""
        gp.tensor_scalar_max(dst, src, 0.0)

    identf = constp.tile([P, P], F32)
    make_identity(nc, identf[:])
    # f32r identity: walrus refuses mixed 32/16-bit matmul inputs, so f32r
    # transposes pair with an f32r identity (1.5 cyc/row vs 2.0 for f32)
    identr = constp.tile([P, P], F32R)
    nc.scalar.activation(identr[:], identf[:], AF.Identity)
    onesf = constp.tile([1, P], F32)
    nc.vector.memset(onesf[:], 1.0)
    ones1 = constp.tile([1, P], MDT)
    nc.scalar.activation(ones1[:], onesf[:], AF.Identity)
    # ones column for partition-axis reduce matmuls over bf16 tiles
    onescol = constp.tile([P, 1], BDT)
    nc.vector.memset(onescol[:], 1.0)
    # fp8 ones column for reduces over fp8 tiles
    onescol8 = constp.tile([P, 1], F8)
    nc.vector.memset(onescol8[:], 1.0)

    p1_f = constp.tile([P, KC], F32)
    nc.sync.dma_start(p1_f[:], p1.rearrange("(k p) o -> p (k o)", p=P))
    p1_t = constp.tile([P, KC], MDT)
    nc.scalar.activation(p1_t[:], p1_f[:], AF.Identity)
    p7c = constp.tile([P, KC, H], F32)
    for k in range(KC):
        nc.sync.dma_start(p7c[:, k, :], p7[k * P:(k + 1) * P, :])

    # p7 transposed once per core (bf16: only feeds bf16 DVE muls)
    p7h = constp.tile([P, KH, C], BDT)
    for j in range(KH):
        ps = psmm.tile([P, 512], F32, tag="g1", name="ps_p7h")
        for i in range(KC):
            nc.tensor.transpose(ps[:, i * P:(i + 1) * P],
                                p7c[:, i, j * P:(j + 1) * P],
                                identf[:])
        nc.scalar.activation(p7h[:, j, :], ps[:], AF.Identity)

    def mm512(lhsT, rhs, consume):
        """out[m,n] = sum_k lhsT[k,m] * rhs[k,n], 512^3, via 4x4 PE matmuls
        (2x2 fp8 DoubleRow when both operands are fp8).  lhsT/rhs are
        [P, 4, 512] chunk-major tiles; consume(m, psum) gets one [P,512]
        chunk at a time (evac scale may vary per chunk)."""
        dr = lhsT.dtype == F8 and rhs.dtype == F8
        for m in range(4):
            ps = psmm.tile([P, 512], F32, tag="g1")
            if dr:
                for kk in range(2):
                    nc.tensor.matmul(ps[:],
                                     lhsT[:, 2 * kk:2 * kk + 2,
                                          m * P:(m + 1) * P],
                                     rhs[:, 2 * kk:2 * kk + 2, :],
                                     perf_mode=PM.DoubleRow,
                                     start=(kk == 0), stop=(kk == 1))
            else:
                for k in range(4):
                    nc.tensor.matmul(ps[:],
                                     lhsT[:, k, m * P:(m + 1) * P],
                                     rhs[:, k, :],
                                     start=(k == 0), stop=(k == 3))
            consume(m, ps)

    def mm512p(lhsT, rhs, dst, func=AF.Identity, scale=1.0):
        """mm512 with a plain float-scale ACT evacuation per chunk."""
        def f(m, ps):
            nc.scalar.activation(dst[:, m, :], ps[:], func, scale=scale)
        mm512(lhsT, rhs, f)

    def evac(dst, scale):
        """Copy psum chunk to dst with a scale: float or [P,KC] tile."""
        def f(m, ps):
            sc = scale[:, m:m + 1] if not isinstance(scale, float) else scale
            nc.scalar.activation(dst[:, m, :], ps[:], AF.Identity, scale=sc)
        return f

    def evac_exp(dst, scale, esum):
        """exp(scale * psum) -> dst chunk m (bf16), row sums -> esum.
        scale: float or [P,KC] tile (per-partition, per-chunk)."""
        def f(m, ps):
            sc = scale[:, m:m + 1] if not isinstance(scale, float) else scale
            if esum is None:
                nc.scalar.activation(dst[:, m, :], ps[:], AF.Exp, scale=sc)
            else:
                nc.scalar.activation(dst[:, m, :], ps[:], AF.Exp, scale=sc,
                                     accum_out=esum[:, m:m + 1])
        return f

    def zrow(expT, zscale, tag):
        """Partition-axis softmax normalizer for a transposed exp tile:
        4 accumulating ones-lhsT reduce matmuls -> [1,512] Z/zscale row
        -> DVE reciprocal.  Runs a full pipeline stage before its column
        conversion (zcol) so no engine ever waits on the chain."""
        ones = onescol8 if expT.dtype == F8 else onescol
        psz = psmisc.tile([1, H], F32, tag="psmisc", name="psz" + tag)
        for k in range(KC):
            nc.tensor.matmul(psz[:], ones[:], expT[:, k, :],
                             start=(k == 0), stop=(k == KC - 1))
        zr = small1.tile([1, H], F32, tag="z" + tag)
        nc.scalar.activation(zr[:], psz[:], AF.Identity, scale=1.0 / zscale)
        rrow = small.tile([1, H], F32, tag="rr" + tag + "row",
                          name="rr" + tag + "row")
        nc.vector.reciprocal(rrow[:], zr[:])
        return rrow

    def zcol(rrow, tag):
        """4 tiny PE transposes: [1,512] reciprocal row -> [P,KC] column
        (zscale already folded by zrow's evacuation)."""
        psc = psmisc.tile([P, KC], F32, tag="psmisc", name="psc" + tag)
        for m in range(KC):
            nc.tensor.transpose(psc[:, m:m + 1],
                                rrow[0:1, m * P:(m + 1) * P],
                                identf[0:1, 0:1])
        col = small.tile([P, KC], F32, tag="rcol" + tag, name="rcol" + tag)
        nc.scalar.activation(col[:], psc[:], AF.Identity)
        return col

    # DRAM scratch pool for the XBAR xh path (hazard-tracked, bufs=2)
    if USE_XHB:
        dscr = ctx.enter_context(
            tc.tile_pool(name="dscr", bufs=2, space="DRAM"))

    # ------------------------------------------------------------------
    # Software pipeline: stage-shifted [S1(k), S2(k-1), S3(k-2)] per step.
    # Engine queues are in-order, so this is what lets the PE run item k's
    # level-1 matmuls while items k-1/k-2 wait on evacuations at their
    # level transitions.  Cross-stage tensors live in the big2 pool.
    # ------------------------------------------------------------------

    def s1(n):
        """Loads, elementwise prep, MM1/MM2/MM3/MM5, Z10 chain."""
        st = {"n": n}
        # Batched loads: ONE dma_start per logical tensor piece (HW pays
        # SWDGE generation + completion-sem per dma_start; CoreSim
        # undercharges this, HW measures it dearly at 13 DMAs/item).
        xc = xpool.tile([P, KC, H], MDT, tag="xc")
        nc.sync.dma_start(
            xc[:], x[n].rearrange("(k p) h -> p k h", p=P).bitcast(MDT))
        # t2 = roll(x, 2, axis=c): row-shifted loads straight from HBM
        t2c = big.tile([P, KC, H], MDT, tag="t2c")
        nc.sync.dma_start(t2c[SHIFT:P, 0, :],
                          x[n][0:P - SHIFT, :].bitcast(MDT))
        nc.sync.dma_start(t2c[0:SHIFT, 0, :],
                          x[n][C - SHIFT:C, :].bitcast(MDT))
        nc.sync.dma_start(
            t2c[:, 1:KC, :],
            x[n][P - SHIFT:KC * P - SHIFT, :]
            .rearrange("(k p) h -> p k h", p=P).bitcast(MDT))

        # t5c = relu(x) (C-layout), needed first for MM1
        t5c = big.tile([P, KC, H], MDT, tag="t5c")
        for k in range(KC):
            relu(t5c[:, k, :], _f(xc[:, k, :]))

        # t1 = p1 . x
        ps_t1 = psmisc.tile([1, H], F32, tag="psmisc")
        for k in range(KC):
            nc.tensor.matmul(ps_t1[:], p1_t[:, k:k + 1], xc[:, k, :],
                             start=(k == 0), stop=(k == KC - 1))
        t1row = small1.tile([1, H], MDT, tag="t1row")
        nc.scalar.activation(t1row[:], ps_t1[:], AF.Identity)

        # x transposed: xh[h, c] (bf16).  Emitted between the t1 matvec
        # and the t1h transposes so the PE covers t1row's ACT evacuation.
        xh = big.tile([P, KH, C], BDT, tag="xh")
        if USE_XHB:
            # casting DMA f32->bf16 into DRAM scratch, then XBAR-transposed
            # reads; xh carries bf16 rounding of x, costs ~1MB/item HBM but
            # zero PE/ACT.
            xd = dscr.tile([C, H], BDT, tag="xd", name="xd")
            nc.gpsimd.dma_start(out=xd, in_=x[n])
            for j in range(KH):
                nc.sync.dma_start_transpose(xh[:, j, :],
                                            xd[:, j * P:(j + 1) * P])
        else:
            for j in range(KH):
                ps = psmm.tile([P, C], F32R, tag="g1", name="ps_xh")
                for i in range(KC):
                    nc.tensor.transpose(ps[:, i * P:(i + 1) * P],
                                        xc[:, i, j * P:(j + 1) * P],
                                        identr[:])
                nc.scalar.activation(xh[:, j, :], ps[:].bitcast(F32),
                                     AF.Identity)

        # t1 as per-partition column [P, KH] (for H-layout bias adds)
        ps_t1h = psmisc.tile([P, KH], F32, tag="psmisc")
        for j in range(KH):
            nc.tensor.transpose(ps_t1h[:, j:j + 1],
                                _f(t1row[0:1, j * P:(j + 1) * P]),
                                identf[0:1, 0:1])
        t1h = small1.tile([P, KH], F32, tag="t1h")
        nc.scalar.activation(t1h[:], ps_t1h[:], AF.Identity)

        # broadcast t1 row across partitions via ones-matmul (for t3c);
        # evacuate to SBUF so t3c's add never reads PSUM from the DVE
        ps_bc = psmisc.tile([P, H], F32, tag="psbc")
        nc.tensor.matmul(ps_bc[:], ones1[:], t1row[:],
                         start=True, stop=True)
        t1bc = small1.tile([P, H], F32, tag="t1bc")
        nc.scalar.activation(t1bc[:], ps_bc[:], AF.Identity)

        # H-layout elementwise (bf16)
        t5h = big.tile([P, KH, C], BDT, tag="t5h")
        t3h = big.tile([P, KH, C], BDT, tag="t3h")
        t4h = big.tile([P, KH, C], BDT, tag="t4h")
        t7h = big.tile([P, KH, C], BDT, tag="t7h")
        for j in range(KH):
            relu(t5h[:, j, :], xh[:, j, :])
            # t3h = roll(x,2,axis=c) + t1  (roll => free-dim shift in H)
            gp.tensor_scalar_add(t3h[:, j, SHIFT:C],
                                 xh[:, j, 0:C - SHIFT],
                                 t1h[:, j:j + 1])
            gp.tensor_scalar_add(t3h[:, j, 0:SHIFT],
                                 xh[:, j, C - SHIFT:C],
                                 t1h[:, j:j + 1])
            nc.vector.tensor_mul(t4h[:, j, :], xh[:, j, :], t3h[:, j, :])
            nc.vector.tensor_mul(t7h[:, j, :], p7h[:, j, :], t5h[:, j, :])

        # C-layout elementwise
        t7c = big2.tile([P, KC, H], BDT, tag="t7c")
        t3c = big.tile([P, KC, H], F32, tag="t3c")
        t4c = big.tile([P, KC, H], F32, tag="t4c")
        t11c = big.tile([P, KC, H], MDT, tag="t11c")
        for k in range(KC):
            nc.vector.tensor_mul(t7c[:, k, :], p7c[:, k, :],
                                 _f(t5c[:, k, :]))
            nc.vector.tensor_add(t3c[:, k, :], _f(t2c[:, k, :]), t1bc[:])
            nc.vector.tensor_mul(t4c[:, k, :], _f(xc[:, k, :]), t3c[:, k, :])
            nc.vector.tensor_tensor(t11c[:, k, :], t4c[:, k, :],
                                    _f(xc[:, k, :]), op=ALU.min)

        # MM1: t8e = exp(t5^T x / sqrt(c)); rows normalized via rr8 at MM5
        t8 = big2.tile([P, KH, H], BDT, tag="t8")
        esum8 = small1.tile([P, KH], F32, tag="esum8")
        mm512(t5c, xc, evac_exp(t8, INV_SC, esum8))
        rr8 = small1.tile([P, KH], F32, tag="rr8")
        nc.vector.reciprocal(rr8[:], esum8[:])

        # MM2: t9_raw = t5^T_h t4_h   (true t9 = raw * inv_sh)   [c, d]
        t9 = big2.tile([P, KC, C], BDT, tag="t9")
        mm512p(t5h, t4h, t9)
        if USE_FP8:
            # fp8 shadow of t9 for MM7 only (t12's path needs bf16)
            t9f8 = big2.tile([P, KC, C], F8, tag="t9f8")
            for k in range(KC):
                nc.vector.tensor_copy(t9f8[:, k, :], t9[:, k, :])
        else:
            t9f8 = t9

        # MM3 (swapped): E10T = exp(t3 t7^T / sqrt(h)) = t10e^T  [d, j]
        e10T = big2.tile([P, KC, C], F8 if USE_FP8 else BDT, tag="e10T")
        mm512p(t3h, t7h, e10T, func=AF.Exp, scale=INV_SH)

        # MM5: t13T_raw = t2^T t11; rows scaled by t8's 1/rowsum   [g, h]
        # Emitted BEFORE the Z10 chain: the ones-reduces wait on E10T's
        # ACT evacs, and the in-order PE queue would stall there otherwise.
        t13T = big2.tile([P, KH, H], BDT, tag="t13T")
        mm512(t2c, t11c, evac(t13T, rr8))

        # Z10[j] = sum_d E10T[d,j] as a [1,512] reciprocal row; the
        # column conversion happens in s2 under MM4's cover.
        rr10row = zrow(e10T, INV_SC * INV_SH, "10")

        st.update(t7c=t7c, t8=t8, t9=t9, t9f8=t9f8, e10T=e10T, t13T=t13T,
                  rr10row=rr10row)
        return st

    def s2(st):
        """Level-2/3 matmuls MM4 (t12), MM7 (E14T), MM6 (t15), Z14 chain."""
        # MM4: t12 = t9^T t7 / (sqrt(c) sqrt(h))         [i, h]
        t12 = big2.tile([P, KC, H], BDT, tag="t12")
        mm512p(st["t9"], st["t7c"], t12, scale=INV_SC * INV_SH)

        # rr10c column conversion under MM4's PE cover: its DVE/ACT
        # inputs were produced a full stage ago, so no engine waits.
        rr10c = zcol(st["rr10row"], "10")

        # MM7 (swapped): E14T = exp((INV_SC*INV_SH/Z10[j]) * (t10e t9))
        # [j, i] - a clean fp8 DoubleRow GEMM; rr10c (t10's normalizer +
        # both scale factors) is ready before the first evacuation.
        e14T = big2.tile([P, KC, C], BDT, tag="e14T")
        mm512(st["e10T"], st["t9f8"], evac_exp(e14T, rr10c, None))

        # MM6: t15e = exp(t13 t8 / (sqrt(c) sqrt(h)))  [i, k]
        # Emitted BEFORE the Z14 chain so the ones-reduces wait behind a
        # full GEMM while MM7's exp evacuations drain on the ACT.
        t15 = big2.tile([P, KH, H], BDT, tag="t15")
        esum15 = small.tile([P, KH], F32, tag="esum15")
        mm512(st["t13T"], st["t8"], evac_exp(t15, INV_SC * INV_SH, esum15))
        rr15 = small.tile([P, KH], F32, tag="rr15")
        nc.vector.reciprocal(rr15[:], esum15[:])
        nc.vector.tensor_scalar_mul(rr15[:], rr15[:], INV_SC)

        # Z14[i] = sum_j E14T[j,i] reciprocal row (column form in s3)
        rr14row = zrow(e14T, INV_SH, "14")

        st.update(t12=t12, e14T=e14T, t15=t15, rr15=rr15, rr14row=rr14row)

    def s3(st):
        """Level-4 matmuls MM8/MM9 and the output store."""
        n = st["n"]
        # rr14c = INV_SH/Z14 rides MM9's evac; the tiny transposes run a
        # full stage after the Z14 row chain, so their inputs are ready.
        rr14c = zcol(st["rr14row"], "14")

        # MM8: t16 = t12^T t14e^T; rows scaled by t15 norm / sqrt(c)
        t16 = big.tile([P, KH, C], BDT, tag="t16")
        mm512(st["t12"], st["e14T"], evac(t16, st["rr15"]))

        # MM9: t17 = t16^T t15e; rows scaled by t14 norm / sqrt(h)
        t17 = big.tile([P, KC, H], F32, tag="t17")
        mm512(t16, st["t15"], evac(t17, rr14c))

        nc.sync.dma_start(
            out[n].rearrange("(k p) h -> p k h", p=P), t17[:])

    states = {}
    ntot = NB * loop
    for step in range(ntot + 2):
        if step < ntot:
            states[step] = s1(step % NB)
        if 1 <= step and step - 1 < ntot:
            s2(states[step - 1])
        if 2 <= step:
            s3(states.pop(step - 2))

    ctx.close()


_NC_CACHE = {}


def _get_program():
    if "nc" not in _NC_CACHE:
        _NC_CACHE["nc"] = build_program()
    return _NC_CACHE["nc"]


def kernel(x, p1_w, p7_w):
    x = np.ascontiguousarray(x, dtype=np.float32)
    p1 = np.ascontiguousarray(p1_w, dtype=np.float32)
    p7 = np.ascontiguousarray(np.asarray(p7_w).reshape(C, H),
                              dtype=np.float32)
    nc = _get_program()
    in_maps = [
        {"x": np.ascontiguousarray(x[i * NB:(i + 1) * NB]),
         "p1": p1, "p7": p7}
        for i in range(NCORES)
    ]
    res = bass_utils.run_bass_kernel_spmd(nc, in_maps,
                                          core_ids=list(range(NCORES)))
    outs = [np.asarray(res.results[i]["out"]) for i in range(NCORES)]
    return np.concatenate(outs, axis=0)


if __name__ == "__main__":
    nc = build_program()
    print("built ok")
